# revision 2
# baseline (speedup 1.0000x reference)
"""Trainium2 Bass kernel for the MetricLearning pairwise loss.

Reference math:
    d2[i,j] = max(||x_i||^2 + ||x_j||^2 - 2 x_i.x_j, EPS)
    a = d2/(2k)/sigma^2 ; b = d2/(2k)/omega^2 ; c1 = k/2-1
    per_pair = same ? (-c1*log(a) + a/2) : (c1*log(b) - b/2)
    loss = sum_{i<j} per_pair

Split: everything linear in d2 has a closed form the host computes exactly
in fp64; the device computes only the two log sums
    S1 = sum_{i<j} ln(d2),   S2 = sum_{same,i<j} ln(d2).

Device pipeline: fp8 DoubleRow matmul chains produce  t = -d2/2  directly
in PSUM -- BOTH norm terms ride inside the contraction as aug features:
features 1020/1021 carry -sq_i/2 on the lhs (rhs side const 4.0), features
1022/1023 carry -sq_j/2 on the rhs (lhs side const 4.0), each as an fp8
hi/lo split at weight 4.  One Ln activation per segment (scale=-2, no
bias) then a DVE add-reduce into a [P,32] accumulator that is DMA'd out
raw; the host does the final 128-way partition sum in fp64.

Rows are globally SORTED BY LABEL (max run <= 128), so same-label pairs
live only inside a 256-block or in the 128-wide corner between
consecutive blocks.  Block diagonals avoid the old full-tile redundancy:
unit u0 computes its block's full [128,256] (triangle via the symmetric
trick + the in-block cross counted once), unit u1 computes only its own
[128,128] triangle.

Sharding: 16 row-blocks of 256; K8 super-node orientation gives every
core 10 resident blocks (identical SPMD program, per-core slab
permutation).  Work is cut into 21 segments of <=512 output columns (one
PSUM bank each, 4 DoubleRow passes), ordered to match the DMA stream,
which runs half-slabs (128 cols) alternating across BOTH hardware DGE
queues (sync + scalar).  A PE warmup burst keeps the HAM activity window
hot so the clock ramps to 2.4 GHz right after the real chains begin.
"""

import numpy as np
import ml_dtypes

N = 4096
D = 1024
P = 128
NB = 16          # row blocks
BLK = 256        # rows per block
KC = D // P      # k chunks (8)
NCORES = 8
NSLOT = 10       # distinct blocks resident per core
NSH = NSLOT * 2  # half-slab count

SIGMA = 0.2
OMEGA = 1.0
K_F = float(N)
C1 = K_F / 2.0 - 1.0                      # 2047
A_C = 1.0 / (2.0 * K_F * SIGMA * SIGMA)
B_C = 1.0 / (2.0 * K_F * OMEGA * OMEGA)
LOG_A = float(np.log(A_C))
LOG_B = float(np.log(B_C))
MARGIN = 128.0   # diag clamp floor; raw diag |d2| < ~50, off-diag > ~1400
LNM = float(np.log(MARGIN))
ACC_W = 32

# K8 super-node orientation: core c owns 3 super-edges (first one is
# c+1 so the consecutive-block corner lands at slot 2), plus one
# crosswise-split super-pair.  Covers all 120 block pairs exactly once.
OWNED = {0: [1, 7, 6], 1: [2, 6, 7], 2: [3, 4, 5], 3: [4, 0, 5],
         4: [5, 0, 1], 5: [6, 0, 1], 6: [7, 3, 2], 7: [2, 3, 4]}
MATCH = {0: 2, 2: 0, 1: 3, 3: 1, 4: 6, 6: 4, 5: 7, 7: 5}

# Segment table: (unit g, sh_start, n_half_slabs, kind, labb_off)
#   kind: 'spec'   256 wide: clamp, Ln, chunked reduce [T(half),C(full)],
#                  mask over 256 -> chunked reduce [M(m_half),M(m_full)]
#         'tri'    128 wide: clamp, Ln, reduce [T(half)], mask -> [M(m_half)]
#         'corner' 512 wide: Ln, reduce [full]; mask cols 0:128 -> [m_full]
#         'norm'   Ln, reduce [full]
SEGS = [
    (0, 0, 2, 'spec', 0),      # A1: u0 x slot0 (b0 tri + in-block cross)
    (1, 1, 1, 'tri', 128),     # B:  u1 x slot0h1 (u1 triangle)
    (2, 2, 2, 'spec', 256),    # C:  u2 x slot1 (b1 tri + in-block cross)
    (3, 3, 1, 'tri', 384),     # D:  u3 x slot1h1
    (0, 2, 2, 'norm', None),   # A2: u0 x slot1
    (1, 2, 4, 'corner', 256),  # E:  u1 x slots1-2 (corner A in cols 0:128)
    (3, 4, 4, 'corner', 512),  # F:  u3 x slots2-3 (corner B in cols 0:128)
    (0, 4, 4, 'norm', None),   # G
    (2, 4, 4, 'norm', None),   # H
    (1, 6, 4, 'norm', None),   # I
    (0, 8, 4, 'norm', None),   # J
    (2, 8, 4, 'norm', None),   # K
    (3, 8, 4, 'norm', None),   # L
    (1, 10, 4, 'norm', None),  # M
    (0, 12, 4, 'norm', None),  # N
    (2, 12, 4, 'norm', None),  # O
    (3, 12, 4, 'norm', None),  # P
    (1, 14, 4, 'norm', None),  # Q
    (0, 16, 2, 'norm', None),  # R
    (2, 18, 2, 'norm', None),  # S
    (3, 18, 2, 'norm', None),  # T
]

# acc-column schema built to mirror the device emission order
_SCHEMA = []
for g, sh0, nsh, kind, off in SEGS:
    if kind == 'spec':
        _SCHEMA += ['half', 'full', 'm_half', 'm_full']
    elif kind == 'tri':
        _SCHEMA += ['half', 'm_half']
    elif kind == 'corner':
        _SCHEMA += ['full', 'm_full']
    else:
        _SCHEMA += ['full']
assert len(_SCHEMA) <= ACC_W, len(_SCHEMA)

# DMA issue order per queue (sync gets even half-slabs, scalar odd ones;
# lhsx rides early on scalar since kp=3 of the first segments needs it)
SYNC_SH = [0, 2, 4, 6, 8, 10, 12, 14, 16, 18]
SCAL_SH = [1, 3, 5, 7, 9, 11, 13, 15, 17, 19]

NWARM = 10       # PE warmup matmuls before real chains


def _core_slabs(d):
    slabs = [2 * d, 2 * d + 1]
    for o in OWNED[d]:
        slabs += [2 * o, 2 * o + 1]
    cp = MATCH[d]
    if d < cp:
        slabs += [2 * cp, 2 * cp + 1]
    else:
        slabs += [2 * cp + 1, 2 * cp]
    assert len(slabs) == NSLOT and len(set(slabs)) == NSLOT
    return slabs


_PROG_CACHE = {}


def _build_program():
    if "nc" in _PROG_CACHE:
        return _PROG_CACHE["nc"]
    import concourse.bass as bass  # noqa: F401
    import concourse.bacc as bacc
    import concourse.mybir as mybir
    import concourse.tile as tile

    F32 = mybir.dt.float32
    BF16 = mybir.dt.bfloat16
    FP8 = mybir.dt.float8e4
    AF = mybir.ActivationFunctionType
    ALU = mybir.AluOpType
    DR = mybir.MatmulPerfMode.DoubleRow

    nc = bacc.Bacc("TRN2", target_bir_lowering=False, debug=False,
                   num_devices=NCORES)
    xtp_d = nc.dram_tensor("xtp", [NSH, P, KC * P], FP8,
                           kind="ExternalInput").ap()
    lhs_d = nc.dram_tensor("lhsx", [P, 4, 2, P], FP8,
                           kind="ExternalInput").ap()
    lab_d = nc.dram_tensor("lab", [1, 640], BF16, kind="ExternalInput").ap()
    rl_d = nc.dram_tensor("rowlab", [P, 4], F32, kind="ExternalInput").ap()
    out_d = nc.dram_tensor("out", [P, ACC_W], F32, kind="ExternalOutput").ap()

    with tile.TileContext(nc) as tc:
        with (
            tc.tile_pool(name="persist", bufs=1) as persist,
            tc.tile_pool(name="ltpool", bufs=4) as ltpool,
            tc.tile_pool(name="tcpool", bufs=2) as tcpool,
            tc.tile_pool(name="psum", bufs=8, space="PSUM") as psum,
        ):
            xall = persist.tile([P, NSH, KC, P], FP8, tag="xall")
            lhsx = persist.tile([P, 4, 2, P], FP8, tag="lhsx")
            labb = persist.tile([P, 640], F32, tag="labb")
            labr = persist.tile([1, 640], BF16, tag="labr")
            rl = persist.tile([P, 4], F32, tag="rl")
            ones2 = persist.tile([2, P], BF16, tag="ones2")
            acc = persist.tile([P, ACC_W], F32, tag="acc")
            maskb = persist.tile([P, 1024], F32, tag="maskb")
            prodb = persist.tile([P, 1024], F32, tag="prodb")
            wm8 = persist.tile([P, 512], FP8, tag="wm8")
            wsink = persist.tile([P, 1], F32, tag="wsink")

            # DMA stream: half-slabs alternate across both HW queues so
            # slab0/slab1 land ~2x earlier than a single-queue stream
            nc.scalar.dma_start(out=labr[:], in_=lab_d[:])
            nc.scalar.dma_start(out=rl[:], in_=rl_d[:])
            nc.sync.dma_start(out=xall[:, SYNC_SH[0]], in_=xtp_d[SYNC_SH[0]])
            nc.scalar.dma_start(out=xall[:, SCAL_SH[0]],
                                in_=xtp_d[SCAL_SH[0]])
            nc.scalar.dma_start(out=lhsx[:], in_=lhs_d[:])
            for sh in SYNC_SH[1:]:
                nc.sync.dma_start(out=xall[:, sh], in_=xtp_d[sh])
            for sh in SCAL_SH[1:]:
                nc.scalar.dma_start(out=xall[:, sh], in_=xtp_d[sh])

            nc.gpsimd.memset(wm8[:], 1.0)
            nc.gpsimd.memset(ones2[:], 1.0)
            nc.gpsimd.memset(acc[:], 0.0)

            # PE warmup: DoubleRow dummies keep the HAM activity window
            # busy from the entry barrier until slab0/1 land, so the 2.4
            # GHz unthrottle fires right after the real chains start
            wlhs = wm8[:, 0:256].rearrange("p (k m) -> p k m", k=2)
            wrhs = wm8[:].rearrange("p (k c) -> p k c", k=2)
            wt = psum.tile([P, 512], F32, tag="seg")
            for i in range(NWARM - 2):
                nc.tensor.matmul(wt[:, 0:256], wlhs, wrhs,
                                 start=True, stop=True, perf_mode=DR)
            nc.vector.tensor_copy(wsink[:], wt[:, 0:1])

            # broadcast the 640-wide label row across partitions via PE
            for lo, w in ((0, 512), (512, 128)):
                pl = psum.tile([P, 512], F32, tag="seg")
                nc.tensor.matmul(pl[:, 0:w], ones2[0:1, :],
                                 labr[0:1, lo:lo + w], start=True, stop=True)
                nc.vector.tensor_copy(labb[:, lo:lo + w], pl[:, 0:w])
            wt2 = psum.tile([P, 512], F32, tag="seg")
            for i in range(2):
                nc.tensor.matmul(wt2[:, 0:256], wlhs, wrhs,
                                 start=True, stop=True, perf_mode=DR)
            nc.vector.tensor_copy(wsink[:], wt2[:, 0:1])

            def mm_chain(t_ap, g, sh0, nsh):
                ls, u = g >> 1, g & 1
                for kp in range(KC // 2):
                    if kp == KC // 2 - 1:
                        lhs = lhsx[:, g, :, :]
                    else:
                        lhs = xall[:, 2 * ls + u, 2 * kp:2 * kp + 2, :]
                    nc.tensor.matmul(
                        t_ap, lhs,
                        xall[:, sh0:sh0 + nsh, 2 * kp:2 * kp + 2, :]
                            .rearrange("p s k c -> p k s c"),
                        start=(kp == 0), stop=(kp == KC // 2 - 1),
                        perf_mode=DR)

            col = [0]
            moff = [0]

            def next_cols(n):
                c = col[0]
                col[0] += n
                return c

            for g, sh0, nsh, kind, off in SEGS:
                w = nsh * P
                tg = psum.tile([P, 512], F32, tag="seg")
                mm_chain(tg[:, 0:w], g, sh0, nsh)
                lt = ltpool.tile([P, 512], F32, tag="lt")
                if kind in ('spec', 'tri'):
                    # clamp d2 >= MARGIN (t = -d2/2, only diag is affected)
                    tcb = tcpool.tile([P, 256], F32, tag="tc")
                    nc.vector.tensor_scalar(tcb[:, 0:w], tg[:, 0:w],
                                            -MARGIN / 2.0, None, ALU.min)
                    nc.scalar.activation(lt[:, 0:w], tcb[:, 0:w], AF.Ln,
                                         scale=-2.0)
                else:
                    nc.scalar.activation(lt[:, 0:w], tg[:, 0:w], AF.Ln,
                                         scale=-2.0)
                # unmasked sums
                if kind == 'spec':
                    c = next_cols(2)
                    nc.vector.tensor_reduce(
                        acc[:, c:c + 2],
                        lt[:, 0:256].rearrange("p (a b) -> p a b", a=2),
                        axis=mybir.AxisListType.X, op=ALU.add)
                elif kind == 'tri':
                    c = next_cols(1)
                    nc.vector.tensor_reduce(
                        acc[:, c:c + 1], lt[:, 0:128],
                        axis=mybir.AxisListType.X, op=ALU.add)
                else:
                    c = next_cols(1)
                    nc.vector.tensor_reduce(
                        acc[:, c:c + 1], lt[:, 0:w],
                        axis=mybir.AxisListType.X, op=ALU.add)
                # same-label masked sums (label-sorted rows: only block
                # diagonals and the 128-wide corners need masks)
                if kind in ('spec', 'tri', 'corner'):
                    mw = 256 if kind == 'spec' else 128
                    mo = moff[0]
                    moff[0] += mw
                    mk = maskb[:, mo:mo + mw]
                    pr = prodb[:, mo:mo + mw]
                    nc.vector.tensor_scalar(mk, labb[:, off:off + mw],
                                            rl[:, g:g + 1], None,
                                            ALU.is_equal)
                    nc.vector.tensor_tensor(pr, mk, lt[:, 0:mw], ALU.mult)
                    if kind == 'spec':
                        c = next_cols(2)
                        nc.vector.tensor_reduce(
                            acc[:, c:c + 2],
                            pr.rearrange("p (a b) -> p a b", a=2),
                            axis=mybir.AxisListType.X, op=ALU.add)
                    else:
                        c = next_cols(1)
                        nc.vector.tensor_reduce(
                            acc[:, c:c + 1], pr,
                            axis=mybir.AxisListType.X, op=ALU.add)

            assert col[0] == len(_SCHEMA), (col[0], len(_SCHEMA))
            nc.sync.dma_start(out=out_d[:], in_=acc[:])

    nc.compile()
    _PROG_CACHE["nc"] = nc
    return nc


def _host_prep(outputs, labels):
    """Sort rows by label, build per-core inputs + exact linear terms."""
    x = np.asarray(outputs, dtype=np.float32)
    lab = np.asarray(labels)
    assert x.shape == (N, D)
    perm = np.argsort(lab, kind="stable")
    xp = x[perm]
    labp = lab[perm].astype(np.float64)

    runs_end = np.empty(N, dtype=np.int64)
    i = 0
    max_run = 0
    while i < N:
        j = i
        while j < N and labp[j] == labp[i]:
            j += 1
        runs_end[i:j] = j
        max_run = max(max_run, j - i)
        i = j
    assert max_run <= P, f"label run {max_run} exceeds corner width"

    xq = xp.astype(ml_dtypes.float8_e4m3)
    # True (unquantized) norms make d2 = sq_i + sq_j - 2*xq_i.xq_j unbiased:
    # the value-error correlation in ||xq||^2 cancels the ||e||^2 term.
    x64 = xp.astype(np.float64)
    sq = (x64 ** 2).sum(axis=1)

    # exact linear terms (fp64 closed form, true values)
    npairs = N * (N - 1) // 2
    ssum = x64.sum(axis=0)
    d2_all = N * sq.sum() - float(ssum @ ssum)
    nsame = 0
    d2_same = 0.0
    i = 0
    while i < N:
        j = int(runs_end[i])
        ng = j - i
        nsame += ng * (ng - 1) // 2
        sg = x64[i:j].sum(axis=0)
        d2_same += ng * sq[i:j].sum() - float(sg @ sg)
        i = j
    host_const = (C1 * npairs * LOG_B - (B_C / 2.0) * d2_all
                  - C1 * (LOG_A + LOG_B) * nsame
                  + ((A_C + B_C) / 2.0) * d2_same)

    # fp8 hi/lo split of -sq/2 at weight 4.0 (e4m3 max 448, -sq/8 ~ -128)
    r0 = (-sq / 8.0).astype(ml_dtypes.float8_e4m3)
    r1 = ((-sq / 2.0 - 4.0 * r0.astype(np.float64)) / 4.0).astype(
        ml_dtypes.float8_e4m3)
    sqq = -8.0 * (r0.astype(np.float64) + r1.astype(np.float64))

    # rhs view: features 1020/1021 const 4.0 (lhs-aug partners), 1022/1023
    # carry -sq_j hi/lo
    xq_rhs = xq.copy()
    xq_rhs[:, 1020] = 4.0
    xq_rhs[:, 1021] = 4.0
    xq_rhs[:, 1022] = r0
    xq_rhs[:, 1023] = r1

    # device diag: d2 = 2*sqq - 2*sum_{f<1020} xq^2 must clamp inside MARGIN
    sq8p = (xq[:, :1020].astype(np.float64) ** 2).sum(axis=1)
    d2diag = 2.0 * sqq - 2.0 * sq8p
    assert np.abs(d2diag).max() < MARGIN - 16, np.abs(d2diag).max()

    xt_q = np.ascontiguousarray(xq_rhs.T)                           # [D, N]

    in_maps = []
    for d in range(NCORES):
        slabs = _core_slabs(d)
        # xtp[sh, p, k*128+c] = xq_rhs[blockrow(sh) + c, k*128 + p]
        cols = np.concatenate(
            [np.arange(b * BLK + 128 * h, b * BLK + 128 * h + 128)
             for b in slabs for h in (0, 1)])
        xtp = np.ascontiguousarray(
            xt_q[:, cols].reshape(KC, P, NSH, P).transpose(2, 1, 0, 3)
            .reshape(NSH, P, KC * P))
        # lhs tensor for the LAST k-pair (chunks 6-7): quantized x features,
        # rows 1020/1021 (chunk 7, partitions 124/125) hold -sq_i hi/lo,
        # rows 1022/1023 (partitions 126/127) hold the aug weight 4.0
        lhsxa = np.empty((P, 4, 2, P), dtype=ml_dtypes.float8_e4m3)
        rowlab = np.zeros((P, 4), dtype=np.float32)
        for g, (slab, u) in enumerate(((0, 0), (0, 1), (1, 0), (1, 1))):
            rows = slabs[slab] * BLK + 128 * u + np.arange(P)
            blk = xq[rows, (KC - 2) * P:].reshape(P, 2, P)
            lhsxa[:, g] = blk.transpose(2, 1, 0)    # [part, chunk, row m]
            lhsxa[124, g, 1, :] = r0[rows]
            lhsxa[125, g, 1, :] = r1[rows]
            rowlab[:, g] = labp[rows]
        lhsxa[126, :, 1, :] = 4.0
        lhsxa[127, :, 1, :] = 4.0
        # label row for slot0(256) | slot1(256) | slot2 first 128
        cols0 = np.concatenate(
            [np.arange(b * BLK, (b + 1) * BLK) for b in slabs[:2]]
            + [np.arange(slabs[2] * BLK, slabs[2] * BLK + 128)])
        labrow = labp[cols0].astype(ml_dtypes.bfloat16)[None, :]   # [1, 640]

        in_maps.append({
            "xtp": xtp,
            "lhsx": np.ascontiguousarray(lhsxa),
            "lab": np.ascontiguousarray(labrow),
            "rowlab": rowlab,
        })
    return in_maps, host_const


def _finalize(host_const, outs_list):
    """Combine per-core raw accumulators [P, ACC_W] with the closed form."""
    total = np.float64(host_const)
    s1 = 0.0
    s2 = 0.0
    for o in outs_list:
        v = np.asarray(o, dtype=np.float64).reshape(P, ACC_W).sum(axis=0)
        for c, kindc in enumerate(_SCHEMA):
            if kindc == 'full':
                s1 += v[c]
            elif kindc == 'half':
                s1 += (v[c] - P * LNM) / 2.0
            elif kindc == 'm_full':
                s2 += v[c]
            else:
                s2 += (v[c] - P * LNM) / 2.0
    total += C1 * s1 - 2.0 * C1 * s2
    return np.asarray(total, dtype=np.float32)


def kernel(**inputs):
    from concourse.bass_utils import run_bass_kernel_spmd
    nc = _build_program()
    in_maps, host_const = _host_prep(inputs["outputs"], inputs["labels"])
    res = run_bass_kernel_spmd(nc, in_maps, core_ids=list(range(NCORES)))
    return _finalize(host_const, [r["out"] for r in res.results])
